# revision 10
# baseline (speedup 1.0000x reference)
"""TRN2 Bass kernel for DCTLayer: out = Re(rfft2(grayscale(x))[:, :28, :28]).

Input x: [512, 3, 224, 224] f32. Output: [512, 1, 28, 28] f32.

Math: only a 28x28 corner of the 2D DFT is needed, so compute it as two
small matmuls per image with the grayscale weights folded into the H-side
DFT constants:

    Xb = x[b] viewed as [672, 224]   (rows = (c,h), cols = w)
    T  = K1^T @ Xb                   [56, 224]   (rows 0:28 cos, 28:56 sin)
    out[k1,k2] = sum_w T_c[k1,w] cos(2pi k2 w/W) - sum_w T_s[k1,w] sin(..)

    K1[f=(c,h), m] = gray_c * cos(2pi m h/224)         m in 0..27
                   = gray_c * sin(2pi (m-28) h/224)    m in 28..55

Stage 1 runs as 6 accumulating matmuls (K=112 chunks of the 672 rows) with
constants stationary and two images in the moving operand (N=448) in
float32r (full PE rate at N>=256, ~1.5e-4 rel err). The [56,448]
intermediate is copied to SBUF, PE-transposed in [56,112] slices, and
stage 2 contracts over w with the W-side constants (fp32, tiny).

Sharding: pure data parallel, 64 images per core across 8 cores.
"""

import numpy as np

B_TOTAL = 512
N_CORES = 8
B_CORE = B_TOTAL // N_CORES  # 64
H = W = 224
KOUT = 28
ROWS = 3 * H       # 672
PCH = 112          # partition chunk for the (c,h) contraction
NCH = ROWS // PCH  # 6
GRAY_W = (0.2989, 0.587, 0.114)

GROUPS = B_CORE // 4  # 16 groups of 4 images


def _dft_constants():
    h = np.arange(H)
    k = np.arange(KOUT)
    ang_h = 2.0 * np.pi * np.outer(h, k) / H  # [224, 28]
    # K1: [672, 56] = gray_c * [cos | sin]
    k1_hw = np.concatenate([np.cos(ang_h), np.sin(ang_h)], axis=1)  # [224, 56]
    k1 = np.tile(k1_hw, (3, 1)) * np.repeat(np.array(GRAY_W), H)[:, None]
    # K2: [224, 56] = [cos | -sin]
    k2 = np.concatenate([np.cos(ang_h), -np.sin(ang_h)], axis=1)
    return k1.astype(np.float32), k2.astype(np.float32)


def _build():
    import concourse.bacc as bacc
    import concourse.mybir as mybir
    import concourse.tile as tile

    f32 = mybir.dt.float32
    f32r = mybir.dt.float32r

    nc = bacc.Bacc("TRN2", target_bir_lowering=False, debug=False,
                   num_devices=N_CORES)

    x_d = nc.dram_tensor("x", [B_CORE, 3, H, W], f32r, kind="ExternalInput")
    k1_d = nc.dram_tensor("k1", [ROWS, 2 * KOUT], f32r, kind="ExternalInput")
    k2_d = nc.dram_tensor("k2", [W, 2 * KOUT], f32, kind="ExternalInput")
    id_d = nc.dram_tensor("ident", [2 * KOUT, 2 * KOUT], f32, kind="ExternalInput")
    out_d = nc.dram_tensor("out", [B_CORE, KOUT, KOUT], f32, kind="ExternalOutput")

    # per-pair HBM view: [pair, p(112), b2(2), j(6), w(224)], flat row = 112*j + p
    # (b2, j) merge into one 12-count dim on both sides -> 3-dim DMA AP
    x_r = (x_d.ap()
           .rearrange("b c h w -> b (c h) w")
           .rearrange("(q b2) (j p) w -> q p b2 j w", b2=2, p=PCH))
    k1_r = k1_d.ap().rearrange("(j p) m -> p j m", p=PCH)   # [112, 6, 56]
    k2_r = k2_d.ap().rearrange("(u p) m -> p u m", p=PCH)   # [112, 2, 56]

    with tile.TileContext(nc) as tc:
        with (
            tc.tile_pool(name="const", bufs=1) as cpool,
            tc.tile_pool(name="xin", bufs=6) as xpool,
            tc.tile_pool(name="t2", bufs=4) as t2pool,
            tc.tile_pool(name="lh", bufs=4) as lpool,
            tc.tile_pool(name="ost", bufs=3) as opool,
            tc.tile_pool(name="ps1", bufs=4, space="PSUM") as ps1,
            tc.tile_pool(name="pst", bufs=2, space="PSUM") as pst,
            tc.tile_pool(name="ps2", bufs=2, space="PSUM") as ps2,
        ):
            # constants on ACT's HWDGE ring so SP's ring starts x loads at t=0
            k1t = cpool.tile([PCH, NCH, 2 * KOUT], f32r, tag="k1")
            nc.scalar.dma_start(k1t[:], k1_r)
            k2t = cpool.tile([PCH, 2, 2 * KOUT], f32, tag="k2")
            nc.scalar.dma_start(k2t[:], k2_r)
            ident = cpool.tile([2 * KOUT, 2 * KOUT], f32, tag="id")
            nc.scalar.dma_start(ident[:], id_d[:])

            for g in range(GROUPS):
                t2s = []
                for p2 in range(2):  # image pair within group
                    xt = xpool.tile([PCH, 2, NCH, W], f32r, tag="xt")
                    nc.sync.dma_start(xt[:], x_r[2 * g + p2])
                    psum1 = ps1.tile([2 * KOUT, 2 * W], f32, tag="ps1")
                    for j in range(NCH):
                        nc.tensor.matmul(
                            psum1[:], k1t[:, j, :], xt[:, :, j, :],
                            start=(j == 0), stop=(j == NCH - 1),
                        )
                    t2 = t2pool.tile([2 * KOUT, 2 * W], f32, tag="t2")
                    nc.vector.tensor_copy(t2[:], psum1[:])
                    t2s.append(t2)

                # transpose the four [56,112] slices per w-half, then stage 2
                lhs = []
                for u in range(2):
                    ptr = pst.tile([PCH, 4 * 2 * KOUT], f32, tag="pst")
                    for i in range(4):
                        src = t2s[i // 2][:, (i % 2) * W + u * PCH:
                                          (i % 2) * W + (u + 1) * PCH]
                        nc.tensor.transpose(
                            ptr[:, i * 2 * KOUT:(i + 1) * 2 * KOUT],
                            src, ident[:],
                        )
                    # reorder (i, s, k) -> (s, i, k) so each cos/sin block
                    # is one contiguous 112-wide weights slice for stage 2
                    lh = lpool.tile([PCH, 2, 4, KOUT], f32, tag="lh")
                    nc.scalar.copy(
                        lh[:].rearrange("p s i k -> p i s k"),
                        ptr[:].rearrange("p (i s k) -> p i s k",
                                         i=4, s=2, k=KOUT),
                    )
                    lhs.append(lh)

                psum2 = ps2.tile([4 * KOUT, KOUT], f32, tag="ps2")
                mm = 0
                for u in range(2):
                    for s in range(2):
                        nc.tensor.matmul(
                            psum2[:], lhs[u][:, s, :, :],
                            k2t[:, u, s * KOUT:(s + 1) * KOUT],
                            start=(mm == 0), stop=(mm == 3),
                        )
                        mm += 1
                ost = opool.tile([4 * KOUT, KOUT], f32, tag="ost")
                nc.scalar.copy(ost[:], psum2[:])
                # output DMA on ACT (HWDGE) so the SP sequencer never blocks
                # waiting for compute between input DMAs
                nc.scalar.dma_start(
                    out_d[4 * g:4 * g + 4].rearrange("b k1 k2 -> (b k1) k2"),
                    ost[:],
                )

    nc.compile()
    return nc


def kernel(x: np.ndarray) -> np.ndarray:
    from concourse.bass_utils import run_bass_kernel_spmd

    assert x.shape == (B_TOTAL, 3, H, W), x.shape
    x = np.ascontiguousarray(x, dtype=np.float32)

    k1, k2 = _dft_constants()
    ident = np.eye(2 * KOUT, dtype=np.float32)

    nc = _build()
    in_maps = [
        {"x": x[i * B_CORE:(i + 1) * B_CORE], "k1": k1, "k2": k2, "ident": ident}
        for i in range(N_CORES)
    ]
    res = run_bass_kernel_spmd(nc, in_maps, core_ids=list(range(N_CORES)))
    out = np.concatenate([r["out"] for r in res.results], axis=0)
    return out[:, None, :, :].astype(np.float32)
